# revision 32
# baseline (speedup 1.0000x reference)
"""Trainium2 Bass kernel for nn_MemTransformerLM (Transformer-XL rel-attention).

v2: causally-truncated, load-balanced attention. Every core runs the same
program (SPMD single NEFF) over three 128-row "slots" whose row content is
core-dependent:
  - big   slot: rows [1952-128*pid, 2080-128*pid), scores over j in [0, 2080)
  - small slot: rows [  32+128*pid,  160+128*pid), scores over j in [0, 1056)
  - micro slot: rows [0, 32) (replicated on all cores; core 0's output used),
                scores over j in [0, 128)
Per-core score columns: 2080+1056+128 = 3264 vs 3*2080 = 6240 for the naive
contiguous-span split -- ~1.9x less attention work, perfectly balanced.

The Transformer-XL _rel_shift uses the padded-DRAM-pitch trick: per (head,
slot) a private DRAM buffer [129, 2081]; E rows are written at row pitch
T+1 = 2081 behind a zero column and read back at row stride T with a
rank-dependent offset, reproducing the reference's wrap semantics (including
the "garbage" upper-triangle values inside the two mem-token corners).
Private per-(head,slot) buffers keep the write->read round trips of
different heads independent so the pipeline overlaps across heads.

Masking: big/micro slots use host-precomputed additive masks. The small slot
needs no mask at all: its masked positions (j > i) always wrap into the
never-written low-column region of its P buffer, which is poisoned once with
-1e30 at kernel start.
"""

import os

import numpy as np
import ml_dtypes

import concourse.bass as bass
import concourse.mybir as mybir
import concourse.tile as tile
from concourse import bacc
from concourse.bass import ds
from concourse.bass_utils import run_bass_kernel_spmd
from concourse.masks import make_identity

BF16 = ml_dtypes.bfloat16
DT = mybir.dt
AF = mybir.ActivationFunctionType
ALU = mybir.AluOpType

N_LAYER = 4
N_HEAD = 8
D_HEAD = 64
D_MODEL = 512
D_INNER = 2048
NMT = 16
T = 2048 + 2 * NMT      # 2080
N_CORES = 8
NEG = -1e30
SCALE = 1.0 / np.sqrt(D_HEAD)
HP = 4                  # 128-partition tiles over D_MODEL

# slots: (q-col base, score width, padded prob width, n j-tiles, E k-window lo)
W_BIG = 2080
W_SMALL = 1056
W_MICRO = 128
PAD_BIG = 17 * 128      # 2176
PAD_SMALL = 9 * 128     # 1152
CH_BIG = [512, 512, 512, 512, 32]
CH_SMALL = [512, 512, 32]
PROWS = 129             # 128 rows + 1 wrap-margin row per (head, slot) buffer
PITCH = T + 1           # 2081
POISON_W = 1025         # small-slot poison columns [0, POISON_W)

NK_TOK = 288            # tokens contributed per core to the allgather (128+128+32)
AG_N = D_MODEL * NK_TOK + NK_TOK * D_MODEL  # kT part + v part, elements


def _row_bases(m):
    return 1952 - 128 * m, 32 + 128 * m  # big, small global row starts


def _host_prep(inputs):
    word_emb = np.asarray(inputs["word_emb"], np.float32)
    mem_tokens = np.asarray(inputs["mem_tokens"], np.float32)
    w = np.transpose(word_emb, (1, 0, 2))[:, 0, :]
    mem = mem_tokens[:, 0, :]
    w_full = np.concatenate([mem, w, mem], axis=0)          # [T, 512]

    inv_freq = 1.0 / (10000.0 ** (np.arange(0, D_MODEL, 2, dtype=np.float32) / D_MODEL))
    pos_seq = np.arange(T - 1, -1, -1.0, dtype=np.float32)
    sinusoid = pos_seq[:, None] * inv_freq[None, :]
    pos_emb = np.concatenate([np.sin(sinusoid), np.cos(sinusoid)], axis=-1)
    posT = np.ascontiguousarray(pos_emb.T).astype(BF16)     # [512, T]

    wqkv = np.asarray(inputs["Wqkv"], np.float32).astype(BF16)
    wr = np.asarray(inputs["Wr"], np.float32).astype(BF16)
    wo = np.asarray(inputs["Wo"], np.float32).astype(BF16)
    w1 = np.asarray(inputs["ffn_W1"], np.float32).astype(BF16)
    w2 = np.asarray(inputs["ffn_W2"], np.float32).astype(BF16)
    # fold the 1/sqrt(d) score scale into q (and its biases)
    rwb = np.ascontiguousarray(
        (np.asarray(inputs["r_w_bias"], np.float32) * SCALE).reshape(-1, 1))
    rrb = np.ascontiguousarray(
        (np.asarray(inputs["r_r_bias"], np.float32) * SCALE).reshape(-1, 1))

    # reference mask, True = masked
    M = np.triu(np.ones((T, T), dtype=bool), k=1)
    M[:NMT, :NMT] = False
    M[-NMT:, -NMT:] = False

    mask_micro = np.zeros((128, W_MICRO), np.float32)
    mask_micro[:32, :] = np.where(M[:32, :W_MICRO], NEG, 0.0)

    per_core = []
    for rank in range(N_CORES):
        rb, rs = _row_bases(rank)
        w0 = np.zeros((384, D_MODEL), np.float32)
        w0[0:128] = w_full[rb : rb + 128]
        w0[128:256] = w_full[rs : rs + 128]
        w0[256:288] = w_full[0:32]
        mask_big = np.where(M[rb : rb + 128, :], NEG, 0.0).astype(BF16)
        per_core.append(
            {
                "w0": w0,
                "posT": posT,
                "wqkv": wqkv,
                "wr": wr,
                "wo": wo,
                "w1": w1,
                "w2": w2,
                "rwb": rwb,
                "rrb": rrb,
                "mask_big": np.ascontiguousarray(mask_big),
                "mask_micro": np.ascontiguousarray(mask_micro.astype(BF16)),
            }
        )

    gb = np.ascontiguousarray(
        np.stack(
            [
                np.broadcast_to(np.asarray(inputs["ln1_scale"], np.float32)[:, None, :],
                                (N_LAYER, 128, D_MODEL)),
                np.broadcast_to(np.asarray(inputs["ln1_bias"], np.float32)[:, None, :],
                                (N_LAYER, 128, D_MODEL)),
                np.broadcast_to(np.asarray(inputs["ln2_scale"], np.float32)[:, None, :],
                                (N_LAYER, 128, D_MODEL)),
                np.broadcast_to(np.asarray(inputs["ln2_bias"], np.float32)[:, None, :],
                                (N_LAYER, 128, D_MODEL)),
            ],
            axis=2,
        ).astype(np.float32)
    )
    b1col = np.ascontiguousarray(
        np.asarray(inputs["ffn_b1"], np.float32).reshape(N_LAYER, D_INNER, 1))
    b2bc = np.ascontiguousarray(
        np.broadcast_to(np.asarray(inputs["ffn_b2"], np.float32)[:, None, :],
                        (N_LAYER, 128, D_MODEL)).copy())
    for pc in per_core:
        pc["gb"] = gb
        pc["b1col"] = b1col
        pc["b2bc"] = b2bc
    return per_core


def _layernorm(nc, sm, out_ap, x, g, b, eps):
    f32 = DT.float32
    stats = sm.tile([128, 6], f32, tag="lnst")
    mv = sm.tile([128, 2], f32, tag="lnmv")
    nc.vector.bn_stats(stats[:], x[:])
    nc.vector.bn_aggr(mv[:], stats[:])
    std = sm.tile([128, 1], f32, tag="lnstd")
    nc.scalar.activation(std[:], mv[:, 1:2], AF.Sqrt, bias=eps, scale=1.0)
    rstd = sm.tile([128, 1], f32, tag="lnrstd")
    nc.vector.reciprocal(rstd[:], std[:])
    if g is None:
        nc.vector.tensor_scalar(
            out=out_ap, in0=x[:], scalar1=mv[:, 0:1], scalar2=rstd[:],
            op0=ALU.subtract, op1=ALU.mult,
        )
    else:
        xn = sm.tile([128, D_MODEL], f32, tag="lnxn")
        nc.vector.tensor_scalar(
            out=xn[:], in0=x[:], scalar1=mv[:, 0:1], scalar2=rstd[:],
            op0=ALU.subtract, op1=ALU.mult,
        )
        nc.vector.tensor_tensor(xn[:], xn[:], g, ALU.mult)
        nc.vector.tensor_tensor(out_ap, xn[:], b, ALU.add)


def _build(trivial_gb=True, trivial_b=True):
    nc = bacc.Bacc("TRN2", num_devices=N_CORES, dynamic_dma_scratch_size=4096)
    f32, bf16 = DT.float32, DT.bfloat16

    w0_t = nc.dram_tensor("w0", [384, D_MODEL], f32, kind="ExternalInput")
    posT_t = nc.dram_tensor("posT", [D_MODEL, T], bf16, kind="ExternalInput")
    wqkv_t = nc.dram_tensor("wqkv", [N_LAYER, D_MODEL, 3 * D_MODEL], bf16, kind="ExternalInput")
    wr_t = nc.dram_tensor("wr", [N_LAYER, D_MODEL, D_MODEL], bf16, kind="ExternalInput")
    wo_t = nc.dram_tensor("wo", [N_LAYER, D_MODEL, D_MODEL], bf16, kind="ExternalInput")
    w1_t = nc.dram_tensor("w1", [N_LAYER, D_MODEL, D_INNER], bf16, kind="ExternalInput")
    w2_t = nc.dram_tensor("w2", [N_LAYER, D_INNER, D_MODEL], bf16, kind="ExternalInput")
    rwb_t = nc.dram_tensor("rwb", [D_MODEL, 1], f32, kind="ExternalInput")
    rrb_t = nc.dram_tensor("rrb", [D_MODEL, 1], f32, kind="ExternalInput")
    maskb_t = nc.dram_tensor("mask_big", [128, W_BIG], bf16, kind="ExternalInput")
    maskm_t = nc.dram_tensor("mask_micro", [128, W_MICRO], bf16, kind="ExternalInput")
    gb_t = None if trivial_gb else nc.dram_tensor(
        "gb", [N_LAYER, 128, 4, D_MODEL], f32, kind="ExternalInput")
    b1_t = b2_t = None
    if not trivial_b:
        b1_t = nc.dram_tensor("b1col", [N_LAYER, D_INNER, 1], f32, kind="ExternalInput")
        b2_t = nc.dram_tensor("b2bc", [N_LAYER, 128, D_MODEL], f32, kind="ExternalInput")
    out_t = nc.dram_tensor("wout", [384, D_MODEL], f32, kind="ExternalOutput")

    # per-(head, slot) rel-shift buffers
    p_big = [nc.dram_tensor(f"pb{h}", [PROWS * PITCH], bf16, kind="Internal")
             for h in range(N_HEAD)]
    p_small = [nc.dram_tensor(f"psm{h}", [PROWS * PITCH], bf16, kind="Internal")
               for h in range(N_HEAD)]
    p_micro = [nc.dram_tensor(f"pmi{h}", [PROWS * PITCH], bf16, kind="Internal")
               for h in range(N_HEAD)]

    ag_in = nc.dram_tensor("ag_in", [AG_N], bf16, kind="Internal")
    ag_out = nc.dram_tensor("ag_out", [N_CORES, AG_N], bf16, kind="Internal",
                            addr_space="Shared")
    rg = [list(range(N_CORES))]

    kv_off = D_MODEL * NK_TOK
    agin_k = ag_in[0:kv_off].rearrange("(a b) -> a b", b=NK_TOK)      # [512, 288]
    agin_v = ag_in[kv_off:].rearrange("(a b) -> a b", b=D_MODEL)      # [288, 512]

    with tile.TileContext(nc, num_cores=N_CORES) as tc:
        pid = nc.sync.partition_id()
        with (
            tc.tile_pool(name="const", bufs=1) as constp,
            tc.tile_pool(name="pers", bufs=1) as pers,
            tc.tile_pool(name="wts", bufs=1) as wts,
            tc.tile_pool(name="kv", bufs=1) as kvp,
            tc.tile_pool(name="posb", bufs=2) as posb,
            tc.tile_pool(name="mid", bufs=2) as mid,
            tc.tile_pool(name="epool", bufs=3) as epool,
            tc.tile_pool(name="bdp", bufs=3) as bdp,
            tc.tile_pool(name="probp", bufs=3) as probp,
            tc.tile_pool(name="ptp", bufs=2) as ptp,
            tc.tile_pool(name="sm", bufs=2) as sm,
            tc.tile_pool(name="ps", bufs=4, space="PSUM") as ps,
            tc.tile_pool(name="pspv", bufs=1, space="PSUM") as pspv,
            tc.tile_pool(name="psff", bufs=1, space="PSUM") as psff,
        ):
            ident = constp.tile([128, 128], f32)
            make_identity(nc, ident[:])

            # ---- init P buffers: zeros everywhere; poison for small slots ----
            zrow = epool.tile([128, W_BIG], bf16, tag="esb")
            nc.vector.memset(zrow[:], 0.0)
            for h in range(N_HEAD):
                for pt in (p_big[h], p_micro[h]):
                    v2 = pt.rearrange("(r c) -> r c", c=PITCH)
                    nc.sync.dma_start(v2[0:128, 0:W_BIG], zrow[:, :])
                    nc.sync.dma_start(v2[0:128, W_BIG:PITCH], zrow[:, 0:1])
                    nc.sync.dma_start(v2[128:129, 0:W_BIG], zrow[:1, :])
                    nc.sync.dma_start(v2[128:129, W_BIG:PITCH], zrow[:1, 0:1])
            prow = epool.tile([128, W_BIG], bf16, tag="esb")
            nc.vector.memset(prow[:], NEG)
            for h in range(N_HEAD):
                v2 = p_small[h].rearrange("(r c) -> r c", c=PITCH)
                nc.sync.dma_start(v2[0:128, 0:POISON_W], prow[:, :POISON_W])
                nc.sync.dma_start(v2[128:129, 0:POISON_W], prow[:1, :POISON_W])

            rwb_sb = pers.tile([128, HP], f32)
            rrb_sb = pers.tile([128, HP], f32)
            for d in range(HP):
                nc.sync.dma_start(rwb_sb[:, d : d + 1], rwb_t[d * 128 : (d + 1) * 128, :])
                nc.sync.dma_start(rrb_sb[:, d : d + 1], rrb_t[d * 128 : (d + 1) * 128, :])

            eps_sb = pers.tile([128, 1], f32)
            nc.vector.memset(eps_sb[:], 1e-5)
            maskb_sb = pers.tile([128, W_BIG], bf16)
            nc.sync.dma_start(maskb_sb[:], maskb_t[:])
            maskm_sb = pers.tile([128, W_MICRO], bf16)
            nc.sync.dma_start(maskm_sb[:], maskm_t[:])
            w_sb = pers.tile([128, 3, D_MODEL], f32)
            for qt in range(3):
                nc.sync.dma_start(w_sb[:, qt, :], w0_t[qt * 128 : (qt + 1) * 128, :])

            for l in range(N_LAYER):
                # ---- layer weights ----
                wqkv_sb = wts.tile([128, HP, 3 * D_MODEL], bf16, tag="wqkv")
                wr_sb = wts.tile([128, HP, D_MODEL], bf16, tag="wrl")
                wo_sb = wts.tile([128, HP, D_MODEL], bf16, tag="wol")
                w1_sb = wts.tile([128, HP, D_INNER], bf16, tag="w1l")
                w2_sb = wts.tile([128, 16, D_MODEL], bf16, tag="w2l")
                for d in range(HP):
                    nc.sync.dma_start(wqkv_sb[:, d, :], wqkv_t[l, d * 128 : (d + 1) * 128, :])
                    nc.sync.dma_start(wr_sb[:, d, :], wr_t[l, d * 128 : (d + 1) * 128, :])
                    nc.sync.dma_start(wo_sb[:, d, :], wo_t[l, d * 128 : (d + 1) * 128, :])
                    nc.sync.dma_start(w1_sb[:, d, :], w1_t[l, d * 128 : (d + 1) * 128, :])
                for d in range(16):
                    nc.sync.dma_start(w2_sb[:, d, :], w2_t[l, d * 128 : (d + 1) * 128, :])
                gb_sb = None
                if not trivial_gb:
                    gb_sb = wts.tile([128, 4, D_MODEL], f32, tag="gbl")
                    nc.sync.dma_start(gb_sb[:], gb_t[l])
                b1_sb = b2_sb = None
                if not trivial_b:
                    b1_sb = wts.tile([128, 16], f32, tag="b1l")
                    for d in range(16):
                        nc.sync.dma_start(b1_sb[:, d : d + 1], b1_t[l, d * 128 : (d + 1) * 128, :])
                    b2_sb = wts.tile([128, D_MODEL], f32, tag="b2l")
                    nc.sync.dma_start(b2_sb[:], b2_t[l])

                # ---- transpose residual -> wT bf16 ----
                wT_sb = wts.tile([128, HP, 384], bf16, tag="wT")
                for qt in range(3):
                    for d in range(HP):
                        pt = ps.tile([128, 512], f32, tag="pp")
                        nc.tensor.transpose(
                            pt[:, :128], w_sb[:, qt, d * 128 : (d + 1) * 128], ident[:]
                        )
                        nc.scalar.copy(wT_sb[:, d, qt * 128 : (qt + 1) * 128], pt[:, :128])

                # ---- q/k projections (q pre-scaled by 1/sqrt(d)) ----
                qwT = wts.tile([128, HP, 384], bf16, tag="qwT")
                qrT = wts.tile([128, HP, 384], bf16, tag="qrT")
                kT_own = wts.tile([128, HP, NK_TOK], bf16, tag="kTown")
                for hp in range(HP):
                    pq = ps.tile([128, 512], f32, tag="pp")
                    for d in range(HP):
                        nc.tensor.matmul(
                            pq[:, :384],
                            wqkv_sb[:, d, hp * 128 : hp * 128 + 128],
                            wT_sb[:, d, :],
                            start=(d == 0), stop=(d == HP - 1),
                        )
                    nc.scalar.activation(
                        qwT[:, hp, :], pq[:, :384], AF.Identity,
                        bias=rwb_sb[:, hp : hp + 1], scale=float(SCALE),
                    )
                    nc.scalar.activation(
                        qrT[:, hp, :], pq[:, :384], AF.Identity,
                        bias=rrb_sb[:, hp : hp + 1], scale=float(SCALE),
                    )
                    pk = ps.tile([128, 512], f32, tag="pp")
                    for d in range(HP):
                        nc.tensor.matmul(
                            pk[:, :384],
                            wqkv_sb[:, d, D_MODEL + hp * 128 : D_MODEL + hp * 128 + 128],
                            wT_sb[:, d, :],
                            start=(d == 0), stop=(d == HP - 1),
                        )
                    nc.scalar.copy(kT_own[:, hp, :], pk[:, :NK_TOK])
                    nc.sync.dma_start(
                        agin_k[hp * 128 : (hp + 1) * 128, :], kT_own[:, hp, :]
                    )
                v_own = wts.tile([128, 3, D_MODEL], bf16, tag="vown")
                for qt in range(3):
                    pv = ps.tile([128, 512], f32, tag="pp")
                    for d in range(HP):
                        nc.tensor.matmul(
                            pv[:],
                            wT_sb[:, d, qt * 128 : (qt + 1) * 128],
                            wqkv_sb[:, d, 2 * D_MODEL :],
                            start=(d == 0), stop=(d == HP - 1),
                        )
                    nc.vector.tensor_copy(v_own[:, qt, :], pv[:])
                    rows = 32 if qt == 2 else 128
                    nc.sync.dma_start(
                        agin_v[qt * 128 : qt * 128 + rows, :], v_own[:rows, qt, :]
                    )

                nc.gpsimd.collective_compute(
                    "AllGather", ALU.bypass, replica_groups=rg,
                    ins=[ag_in[:]], outs=[ag_out[:]],
                )

                # ---- rT (posT streamed once per layer) ----
                rT_sb = wts.tile([128, HP, T], bf16, tag="rT")
                for ch in range(5):
                    cw = CH_BIG[ch]
                    c0 = ch * 512
                    pos_ch = posb.tile([128, HP, 512], bf16, tag="posch")
                    for d in range(HP):
                        nc.sync.dma_start(
                            pos_ch[:, d, :cw],
                            posT_t[d * 128 : (d + 1) * 128, c0 : c0 + cw],
                        )
                    for hp in range(HP):
                        pr = ps.tile([128, 512], f32, tag="pp")
                        for d in range(HP):
                            nc.tensor.matmul(
                                pr[:, :cw],
                                wr_sb[:, d, hp * 128 : hp * 128 + 128],
                                pos_ch[:, d, :cw],
                                start=(d == 0), stop=(d == HP - 1),
                            )
                        nc.scalar.copy(rT_sb[:, hp, c0 : c0 + cw], pr[:, :cw])

                # ---- gathered K/V into SBUF ----
                kT_all = kvp.tile([128, HP, T], bf16, tag="kTall")
                v_all = kvp.tile([128, 17, D_MODEL], bf16, tag="vall")
                nc.vector.memset(v_all[:, 16, :], 0.0)
                for r in range(N_CORES):
                    rb, rs = _row_bases(r)
                    srck = ag_out[r, 0:kv_off].rearrange("(a b) -> a b", b=NK_TOK)
                    srcv = ag_out[r, kv_off:].rearrange("(a b) -> a b", b=D_MODEL)
                    for hp in range(HP):
                        nc.sync.dma_start(
                            kT_all[:, hp, rb : rb + 128],
                            srck[hp * 128 : (hp + 1) * 128, 0:128],
                        )
                        nc.sync.dma_start(
                            kT_all[:, hp, rs : rs + 128],
                            srck[hp * 128 : (hp + 1) * 128, 128:256],
                        )
                        if r == 0:
                            nc.sync.dma_start(
                                kT_all[:, hp, 0:32],
                                srck[hp * 128 : (hp + 1) * 128, 256:288],
                            )
                    # v rows: big -> [rb, rb+128), small -> [rs, rs+128), micro r0
                    for base, lo in ((0, rb), (128, rs)):
                        t0_, p0 = lo // 128, lo % 128
                        n1 = 128 - p0
                        nc.sync.dma_start(
                            v_all[p0:128, t0_, :], srcv[base : base + n1, :]
                        )
                        nc.sync.dma_start(
                            v_all[0 : 128 - n1, t0_ + 1, :],
                            srcv[base + n1 : base + 128, :],
                        )
                    if r == 0:
                        nc.sync.dma_start(v_all[0:32, 0, :], srcv[256:288, :])

                # ---- attention ----
                # slot parameters: (qcol0, width, padded width, chunks,
                #                   k-window lo, p-tensor list, probT tile0, n tiles)
                def slot_params(h, si):
                    if si == 0:
                        return (0, W_BIG, PAD_BIG, CH_BIG, 0, p_big[h],
                                128 + pid * 128, 0, 17)
                    if si == 1:
                        return (128, W_SMALL, PAD_SMALL, CH_SMALL, T - W_SMALL,
                                p_small[h], 2048 - pid * 128, 17, 9)
                    return (256, W_MICRO, W_MICRO, [128], T - W_MICRO,
                            p_micro[h], 2080, 26, 1)

                # E computation for one head -> DRAM (decoupled from the
                # rel-shift round trip; interleaved with pass2 below)
                def e_pass(h):
                    hp, par = h // 2, (h % 2) * 64
                    for si in range(3):
                        qc0, wj, wpad, chs, klo, pt_t, off, jt0, njt = slot_params(h, si)
                        p2d = pt_t.rearrange("(r c) -> r c", c=PITCH)
                        qsl = slice(qc0, qc0 + 128)
                        e_sb = epool.tile([128, W_BIG], bf16, tag="esb")
                        cpos = 0
                        for cw in chs:
                            pe = ps.tile([128, 512], f32, tag="pp")
                            nc.tensor.matmul(
                                pe[:, :cw],
                                qrT[par : par + 64, hp, qsl],
                                rT_sb[par : par + 64, hp, klo + cpos : klo + cpos + cw],
                                start=True, stop=True,
                            )
                            nc.scalar.copy(e_sb[:, cpos : cpos + cw], pe[:, :cw])
                            cpos += cw
                        if si == 2:
                            # corner-garbage columns k in [0, 16)
                            pe = ps.tile([128, 512], f32, tag="pp")
                            nc.tensor.matmul(
                                pe[:, :16],
                                qrT[par : par + 64, hp, qsl],
                                rT_sb[par : par + 64, hp, 0:16],
                                start=True, stop=True,
                            )
                            ec = mid.tile([128, 16], bf16, tag="ecrn")
                            nc.scalar.copy(ec[:], pe[:, :16])
                            nc.scalar.dma_start(p2d[0:128, 1:17], ec[:])
                        nc.scalar.dma_start(
                            p2d[0:128, 1 + klo : 1 + klo + wj], e_sb[:, :wj]
                        )

                # pass 2: shifted read-back, scores, softmax, PV
                attnT = wts.tile([128, HP, 384], bf16, tag="attnT")

                def pass2(h):
                    hp, par = h // 2, (h % 2) * 64
                    ppv = pspv.tile([64, 384], f32, tag="ppv")
                    probT = ptp.tile([128, 27, 128], bf16, tag="probT")
                    for si in range(3):
                        qc0, wj, wpad, chs, klo, pt_t, off, jt0, njt = slot_params(h, si)
                        qsl = slice(qc0, qc0 + 128)

                        bd_sb = bdp.tile([128, W_BIG], bf16, tag="bdsb")
                        src_ap = pt_t[ds(off, 128 * T)].rearrange("(a b) -> a b", b=T)
                        nc.sync.dma_start(bd_sb[:, :wj], src_ap[:, :wj])

                        # scores: AC + BD (+ mask for big/micro; small is
                        # handled entirely by the poison region)
                        if si == 0:
                            nc.vector.tensor_tensor(
                                bd_sb[:, :wj], bd_sb[:, :wj], maskb_sb[:, :wj], ALU.add
                            )
                        elif si == 2:
                            nc.vector.tensor_tensor(
                                bd_sb[:, :wj], bd_sb[:, :wj], maskm_sb[:, :wj], ALU.add
                            )
                        cpos = 0
                        for cw in chs:
                            jsl = slice(cpos, cpos + cw)
                            pa = ps.tile([128, 512], f32, tag="pp")
                            nc.tensor.matmul(
                                pa[:, :cw],
                                qwT[par : par + 64, hp, qsl],
                                kT_all[par : par + 64, hp, jsl],
                                start=True, stop=True,
                            )
                            nc.vector.scalar_tensor_tensor(
                                bd_sb[:, jsl], pa[:, :cw], 1.0,
                                bd_sb[:, jsl], ALU.mult, ALU.add,
                            )
                            cpos += cw

                        # softmax over computed j range
                        prob = probp.tile([128, PAD_BIG], bf16, tag="prob")
                        denom = sm.tile([128, 1], f32, tag="denom")
                        if wpad > wj:
                            nc.vector.memset(prob[:, wj:wpad], 0.0)
                        nc.scalar.activation(
                            prob[:, :wj], bd_sb[:, :wj], AF.Exp,
                            bias=0.0, scale=1.0, accum_out=denom[:, :],
                        )
                        rden = sm.tile([128, 1], f32, tag="rden")
                        nc.vector.reciprocal(rden[:], denom[:])
                        nc.vector.tensor_scalar(
                            out=prob[:, :wpad], in0=prob[:, :wpad],
                            scalar1=rden[:], scalar2=None, op0=ALU.mult,
                        )
                        nc.scalar.dma_start_transpose(
                            probT[:, jt0 : jt0 + njt, :], prob[:, :wpad]
                        )
                        # PV for this slot
                        for t in range(njt):
                            nc.tensor.matmul(
                                ppv[:, qc0 : qc0 + 128],
                                v_all[:, t, h * 64 : h * 64 + 64],
                                probT[:, jt0 + t, :],
                                start=(t == 0), stop=(t == njt - 1),
                            )
                    nc.scalar.copy(attnT[par : par + 64, hp, :], ppv[:])

                for h in range(N_HEAD):
                    e_pass(h)
                for h in range(N_HEAD):
                    pass2(h)

                # ---- Wo + residual + LN1 ----
                for qt in range(3):
                    pw = ps.tile([128, 512], f32, tag="pp")
                    for d in range(HP):
                        nc.tensor.matmul(
                            pw[:],
                            attnT[:, d, qt * 128 : (qt + 1) * 128],
                            wo_sb[:, d, :],
                            start=(d == 0), stop=(d == HP - 1),
                        )
                    x = sm.tile([128, D_MODEL], f32, tag="xres")
                    nc.vector.tensor_tensor(x[:], w_sb[:, qt, :], pw[:], ALU.add)
                    _layernorm(
                        nc, sm, w_sb[:, qt, :], x,
                        None if trivial_gb else gb_sb[:, 0, :],
                        None if trivial_gb else gb_sb[:, 1, :],
                        eps_sb[:],
                    )

                # ---- FFN ----
                w1T = wts.tile([128, HP, 384], bf16, tag="wT")
                for qt in range(3):
                    for d in range(HP):
                        pt = ps.tile([128, 512], f32, tag="pp")
                        nc.tensor.transpose(
                            pt[:, :128], w_sb[:, qt, d * 128 : (d + 1) * 128], ident[:]
                        )
                        nc.scalar.copy(w1T[:, d, qt * 128 : (qt + 1) * 128], pt[:, :128])
                pf = [
                    psff.tile([128, 512], f32, tag=f"pf{qt}", name=f"pf{qt}")
                    for qt in range(3)
                ]
                for di in range(16):
                    phh = ps.tile([128, 512], f32, tag="pp")
                    for d in range(HP):
                        nc.tensor.matmul(
                            phh[:, :384],
                            w1_sb[:, d, di * 128 : (di + 1) * 128],
                            w1T[:, d, :],
                            start=(d == 0), stop=(d == HP - 1),
                        )
                    h1t = mid.tile([128, 384], bf16, tag="h1t")
                    if trivial_b:
                        nc.scalar.activation(
                            h1t[:], phh[:, :384], AF.Relu, bias=0.0, scale=1.0
                        )
                    else:
                        nc.scalar.activation(
                            h1t[:], phh[:, :384], AF.Relu,
                            bias=b1_sb[:, di : di + 1], scale=1.0,
                        )
                    for qt in range(3):
                        nc.tensor.matmul(
                            pf[qt][:],
                            h1t[:, qt * 128 : (qt + 1) * 128],
                            w2_sb[:, di, :],
                            start=(di == 0), stop=(di == 15),
                        )
                for qt in range(3):
                    x = sm.tile([128, D_MODEL], f32, tag="xres")
                    if trivial_b:
                        nc.vector.tensor_tensor(x[:], pf[qt][:], w_sb[:, qt, :], ALU.add)
                    else:
                        nc.vector.scalar_tensor_tensor(
                            x[:], pf[qt][:], 1.0, b2_sb[:], ALU.mult, ALU.add
                        )
                        nc.vector.tensor_tensor(x[:], x[:], w_sb[:, qt, :], ALU.add)
                    _layernorm(
                        nc, sm, w_sb[:, qt, :], x,
                        None if trivial_gb else gb_sb[:, 2, :],
                        None if trivial_gb else gb_sb[:, 3, :],
                        eps_sb[:],
                    )

            for qt in range(3):
                nc.sync.dma_start(
                    out_t[qt * 128 : (qt + 1) * 128, :], w_sb[:, qt, :]
                )

    nc.compile()
    return nc


_NC_CACHE = {}
LAST_RESULT = None


def kernel(**inputs):
    global LAST_RESULT
    trivial_gb = (
        np.all(np.asarray(inputs["ln1_scale"]) == 1.0)
        and np.all(np.asarray(inputs["ln2_scale"]) == 1.0)
        and np.all(np.asarray(inputs["ln1_bias"]) == 0.0)
        and np.all(np.asarray(inputs["ln2_bias"]) == 0.0)
    )
    trivial_b = (
        np.all(np.asarray(inputs["ffn_b1"]) == 0.0)
        and np.all(np.asarray(inputs["ffn_b2"]) == 0.0)
    )
    per_core = _host_prep(inputs)
    drop = []
    if trivial_gb:
        drop.append("gb")
    if trivial_b:
        drop += ["b1col", "b2bc"]
    for pc in per_core:
        for k in drop:
            pc.pop(k, None)
    key = (trivial_gb, trivial_b)
    if key not in _NC_CACHE:
        _NC_CACHE[key] = _build(trivial_gb=trivial_gb, trivial_b=trivial_b)
    res = run_bass_kernel_spmd(
        _NC_CACHE[key], [dict(pc) for pc in per_core], core_ids=list(range(N_CORES)),
        tmpdir=os.environ.get("BASS_TMPDIR") or None,
    )
    LAST_RESULT = res
    out = np.zeros((T, D_MODEL), np.float32)
    for m in range(N_CORES):
        rb, rs = _row_bases(m)
        wout = res.results[m]["wout"]
        out[rb : rb + 128] = wout[0:128]
        out[rs : rs + 128] = wout[128:256]
        if m == 0:
            out[0:32] = wout[256:288]
    return np.ascontiguousarray(out[:, None, :].astype(np.float32))


# revision 40
# speedup vs baseline: 1.1870x; 1.1870x over previous
"""Trainium2 Bass kernel for nn_MemTransformerLM (Transformer-XL rel-attention).

v2: causally-truncated, load-balanced attention. Every core runs the same
program (SPMD single NEFF) over three 128-row "slots" whose row content is
core-dependent:
  - big   slot: rows [1952-128*pid, 2080-128*pid), scores over j in [0, 2080)
  - small slot: rows [  32+128*pid,  160+128*pid), scores over j in [0, 1056)
  - micro slot: rows [0, 32) (replicated on all cores; core 0's output used),
                scores over j in [0, 128)
Per-core score columns: 2080+1056+128 = 3264 vs 3*2080 = 6240 for the naive
contiguous-span split -- ~1.9x less attention work, perfectly balanced.

The Transformer-XL _rel_shift uses the padded-DRAM-pitch trick: per (head,
slot) a private DRAM buffer [129, 2081]; E rows are written at row pitch
T+1 = 2081 behind a zero column and read back at row stride T with a
rank-dependent offset, reproducing the reference's wrap semantics (including
the "garbage" upper-triangle values inside the two mem-token corners).
Private per-(head,slot) buffers keep the write->read round trips of
different heads independent so the pipeline overlaps across heads.

Masking: big/micro slots use host-precomputed additive masks. The small slot
needs no mask at all: its masked positions (j > i) always wrap into the
never-written low-column region of its P buffer, which is poisoned once with
-1e30 at kernel start.
"""

import os

import numpy as np
import ml_dtypes

import concourse.bass as bass
import concourse.mybir as mybir
import concourse.tile as tile
from concourse import bacc
from concourse.bass import ds
from concourse.bass_utils import run_bass_kernel_spmd
from concourse.masks import make_identity

BF16 = ml_dtypes.bfloat16
DT = mybir.dt
AF = mybir.ActivationFunctionType
ALU = mybir.AluOpType

N_LAYER = 4
N_HEAD = 8
D_HEAD = 64
D_MODEL = 512
D_INNER = 2048
NMT = 16
T = 2048 + 2 * NMT      # 2080
N_CORES = 8
NEG = -1e30
SCALE = 1.0 / np.sqrt(D_HEAD)
HP = 4                  # 128-partition tiles over D_MODEL

# slots: (q-col base, score width, padded prob width, n j-tiles, E k-window lo)
W_BIG = 2080
W_SMALL = 1056
W_MICRO = 128
PAD_BIG = 17 * 128      # 2176
PAD_SMALL = 9 * 128     # 1152
CH_BIG = [512, 512, 512, 512, 32]
CH_SMALL = [512, 512, 32]
PROWS = 129             # 128 rows + 1 wrap-margin row per (head, slot) buffer
PITCH = T + 1           # 2081
POISON_W = 1025         # small-slot poison columns [0, POISON_W)
MASKB_W = 1152          # big-slot mask covers j in [W_BIG-MASKB_W, W_BIG) only

NK_TOK = 288            # tokens contributed per core to the allgather (128+128+32)
AG_N = D_MODEL * NK_TOK + NK_TOK * D_MODEL  # kT part + v part, elements


def _row_bases(m):
    return 1952 - 128 * m, 32 + 128 * m  # big, small global row starts


def _host_prep(inputs):
    word_emb = np.asarray(inputs["word_emb"], np.float32)
    mem_tokens = np.asarray(inputs["mem_tokens"], np.float32)
    w = np.transpose(word_emb, (1, 0, 2))[:, 0, :]
    mem = mem_tokens[:, 0, :]
    w_full = np.concatenate([mem, w, mem], axis=0)          # [T, 512]

    inv_freq = 1.0 / (10000.0 ** (np.arange(0, D_MODEL, 2, dtype=np.float32) / D_MODEL))
    pos_seq = np.arange(T - 1, -1, -1.0, dtype=np.float32)
    sinusoid = pos_seq[:, None] * inv_freq[None, :]
    pos_emb = np.concatenate([np.sin(sinusoid), np.cos(sinusoid)], axis=-1)
    posT = np.ascontiguousarray(pos_emb.T).astype(BF16)     # [512, T]

    wqkv = np.asarray(inputs["Wqkv"], np.float32).astype(BF16)
    wr = np.asarray(inputs["Wr"], np.float32).astype(BF16)
    wo = np.asarray(inputs["Wo"], np.float32).astype(BF16)
    w1 = np.asarray(inputs["ffn_W1"], np.float32).astype(BF16)
    w2 = np.asarray(inputs["ffn_W2"], np.float32).astype(BF16)
    # fold the 1/sqrt(d) score scale into q (and its biases)
    rwb = np.ascontiguousarray(
        (np.asarray(inputs["r_w_bias"], np.float32) * SCALE).reshape(-1, 1))
    rrb = np.ascontiguousarray(
        (np.asarray(inputs["r_r_bias"], np.float32) * SCALE).reshape(-1, 1))

    # reference mask, True = masked
    M = np.triu(np.ones((T, T), dtype=bool), k=1)
    M[:NMT, :NMT] = False
    M[-NMT:, -NMT:] = False

    mask_micro = np.zeros((128, W_MICRO), np.float32)
    mask_micro[:32, :] = np.where(M[:32, :W_MICRO], NEG, 0.0)

    per_core = []
    for rank in range(N_CORES):
        rb, rs = _row_bases(rank)
        w0 = np.zeros((384, D_MODEL), np.float32)
        w0[0:128] = w_full[rb : rb + 128]
        w0[128:256] = w_full[rs : rs + 128]
        w0[256:288] = w_full[0:32]
        mask_big = np.where(M[rb : rb + 128, W_BIG - MASKB_W :], NEG, 0.0).astype(BF16)
        assert not M[rb : rb + 128, : W_BIG - MASKB_W].any()
        per_core.append(
            {
                "w0": w0,
                "posT": posT,
                "wqkv": wqkv,
                "wr": wr,
                "wo": wo,
                "w1": w1,
                "w2": w2,
                "rwb": rwb,
                "rrb": rrb,
                "mask_big": np.ascontiguousarray(mask_big),
                "mask_micro": np.ascontiguousarray(mask_micro.astype(BF16)),
            }
        )

    gb = np.ascontiguousarray(
        np.stack(
            [
                np.broadcast_to(np.asarray(inputs["ln1_scale"], np.float32)[:, None, :],
                                (N_LAYER, 128, D_MODEL)),
                np.broadcast_to(np.asarray(inputs["ln1_bias"], np.float32)[:, None, :],
                                (N_LAYER, 128, D_MODEL)),
                np.broadcast_to(np.asarray(inputs["ln2_scale"], np.float32)[:, None, :],
                                (N_LAYER, 128, D_MODEL)),
                np.broadcast_to(np.asarray(inputs["ln2_bias"], np.float32)[:, None, :],
                                (N_LAYER, 128, D_MODEL)),
            ],
            axis=2,
        ).astype(np.float32)
    )
    b1col = np.ascontiguousarray(
        np.asarray(inputs["ffn_b1"], np.float32).reshape(N_LAYER, D_INNER, 1))
    b2bc = np.ascontiguousarray(
        np.broadcast_to(np.asarray(inputs["ffn_b2"], np.float32)[:, None, :],
                        (N_LAYER, 128, D_MODEL)).copy())
    for pc in per_core:
        pc["gb"] = gb
        pc["b1col"] = b1col
        pc["b2bc"] = b2bc
    return per_core


def _layernorm(nc, sm, out_ap, x, g, b, eps):
    f32 = DT.float32
    stats = sm.tile([128, 6], f32, tag="lnst")
    mv = sm.tile([128, 2], f32, tag="lnmv")
    nc.vector.bn_stats(stats[:], x[:])
    nc.vector.bn_aggr(mv[:], stats[:])
    std = sm.tile([128, 1], f32, tag="lnstd")
    nc.scalar.activation(std[:], mv[:, 1:2], AF.Sqrt, bias=eps, scale=1.0)
    rstd = sm.tile([128, 1], f32, tag="lnrstd")
    nc.vector.reciprocal(rstd[:], std[:])
    if g is None:
        nc.vector.tensor_scalar(
            out=out_ap, in0=x[:], scalar1=mv[:, 0:1], scalar2=rstd[:],
            op0=ALU.subtract, op1=ALU.mult,
        )
    else:
        xn = sm.tile([128, D_MODEL], f32, tag="lnxn")
        nc.vector.tensor_scalar(
            out=xn[:], in0=x[:], scalar1=mv[:, 0:1], scalar2=rstd[:],
            op0=ALU.subtract, op1=ALU.mult,
        )
        nc.vector.tensor_tensor(xn[:], xn[:], g, ALU.mult)
        nc.vector.tensor_tensor(out_ap, xn[:], b, ALU.add)


def _build(trivial_gb=True, trivial_b=True):
    nc = bacc.Bacc("TRN2", num_devices=N_CORES, dynamic_dma_scratch_size=4096)
    f32, bf16 = DT.float32, DT.bfloat16

    w0_t = nc.dram_tensor("w0", [384, D_MODEL], f32, kind="ExternalInput")
    posT_t = nc.dram_tensor("posT", [D_MODEL, T], bf16, kind="ExternalInput")
    wqkv_t = nc.dram_tensor("wqkv", [N_LAYER, D_MODEL, 3 * D_MODEL], bf16, kind="ExternalInput")
    wr_t = nc.dram_tensor("wr", [N_LAYER, D_MODEL, D_MODEL], bf16, kind="ExternalInput")
    wo_t = nc.dram_tensor("wo", [N_LAYER, D_MODEL, D_MODEL], bf16, kind="ExternalInput")
    w1_t = nc.dram_tensor("w1", [N_LAYER, D_MODEL, D_INNER], bf16, kind="ExternalInput")
    w2_t = nc.dram_tensor("w2", [N_LAYER, D_INNER, D_MODEL], bf16, kind="ExternalInput")
    rwb_t = nc.dram_tensor("rwb", [D_MODEL, 1], f32, kind="ExternalInput")
    rrb_t = nc.dram_tensor("rrb", [D_MODEL, 1], f32, kind="ExternalInput")
    maskb_t = nc.dram_tensor("mask_big", [128, MASKB_W], bf16, kind="ExternalInput")
    maskm_t = nc.dram_tensor("mask_micro", [128, W_MICRO], bf16, kind="ExternalInput")
    gb_t = None if trivial_gb else nc.dram_tensor(
        "gb", [N_LAYER, 128, 4, D_MODEL], f32, kind="ExternalInput")
    b1_t = b2_t = None
    if not trivial_b:
        b1_t = nc.dram_tensor("b1col", [N_LAYER, D_INNER, 1], f32, kind="ExternalInput")
        b2_t = nc.dram_tensor("b2bc", [N_LAYER, 128, D_MODEL], f32, kind="ExternalInput")
    out_t = nc.dram_tensor("wout", [384, D_MODEL], f32, kind="ExternalOutput")

    # per-(head, slot) rel-shift buffers
    p_big = [nc.dram_tensor(f"pb{h}", [PROWS * PITCH], bf16, kind="Internal")
             for h in range(N_HEAD)]
    p_small = [nc.dram_tensor(f"psm{h}", [PROWS * PITCH], bf16, kind="Internal")
               for h in range(N_HEAD)]
    p_micro = [nc.dram_tensor(f"pmi{h}", [PROWS * PITCH], bf16, kind="Internal")
               for h in range(N_HEAD)]

    ag_in = nc.dram_tensor("ag_in", [AG_N], bf16, kind="Internal")
    ag_out = nc.dram_tensor("ag_out", [N_CORES, AG_N], bf16, kind="Internal",
                            addr_space="Shared")
    rg = [list(range(N_CORES))]

    kv_off = D_MODEL * NK_TOK
    agin_k = ag_in[0:kv_off].rearrange("(a b) -> a b", b=NK_TOK)      # [512, 288]
    agin_v = ag_in[kv_off:].rearrange("(a b) -> a b", b=D_MODEL)      # [288, 512]

    with tile.TileContext(nc, num_cores=N_CORES) as tc:
        pid = nc.sync.partition_id()
        with (
            tc.tile_pool(name="const", bufs=1) as constp,
            tc.tile_pool(name="pers", bufs=1) as pers,
            tc.tile_pool(name="wts", bufs=1) as wts,
            tc.tile_pool(name="kv", bufs=1) as kvp,
            tc.tile_pool(name="posb", bufs=2) as posb,
            tc.tile_pool(name="mid", bufs=2) as mid,
            tc.tile_pool(name="epool", bufs=3) as epool,
            tc.tile_pool(name="bdp", bufs=2) as bdp,
            tc.tile_pool(name="probp", bufs=2) as probp,
            tc.tile_pool(name="ptp", bufs=2) as ptp,
            tc.tile_pool(name="sm", bufs=2) as sm,
            tc.tile_pool(name="ps", bufs=4, space="PSUM") as ps,
            tc.tile_pool(name="pspv", bufs=1, space="PSUM") as pspv,
            tc.tile_pool(name="psff", bufs=1, space="PSUM") as psff,
        ):
            ident = constp.tile([128, 128], f32)
            make_identity(nc, ident[:])

            # ---- init P buffers: zeros everywhere; poison for small slots ----
            zrow = epool.tile([128, W_BIG], bf16, tag="esb")
            nc.vector.memset(zrow[:], 0.0)
            for h in range(N_HEAD):
                for pt in (p_big[h], p_micro[h]):
                    v2 = pt.rearrange("(r c) -> r c", c=PITCH)
                    nc.sync.dma_start(v2[0:128, 0:W_BIG], zrow[:, :])
                    nc.sync.dma_start(v2[0:128, W_BIG:PITCH], zrow[:, 0:1])
                    nc.sync.dma_start(v2[128:129, 0:W_BIG], zrow[:1, :])
                    nc.sync.dma_start(v2[128:129, W_BIG:PITCH], zrow[:1, 0:1])
            prow = epool.tile([128, W_BIG], bf16, tag="esb")
            nc.vector.memset(prow[:], NEG)
            for h in range(N_HEAD):
                v2 = p_small[h].rearrange("(r c) -> r c", c=PITCH)
                nc.sync.dma_start(v2[0:128, 0:POISON_W], prow[:, :POISON_W])
                nc.sync.dma_start(v2[128:129, 0:POISON_W], prow[:1, :POISON_W])

            rwb_sb = pers.tile([128, HP], f32)
            rrb_sb = pers.tile([128, HP], f32)
            for d in range(HP):
                nc.sync.dma_start(rwb_sb[:, d : d + 1], rwb_t[d * 128 : (d + 1) * 128, :])
                nc.sync.dma_start(rrb_sb[:, d : d + 1], rrb_t[d * 128 : (d + 1) * 128, :])

            eps_sb = pers.tile([128, 1], f32)
            nc.vector.memset(eps_sb[:], 1e-5)
            maskb_sb = pers.tile([128, MASKB_W], bf16)
            nc.sync.dma_start(maskb_sb[:], maskb_t[:])
            maskm_sb = pers.tile([128, W_MICRO], bf16)
            nc.sync.dma_start(maskm_sb[:], maskm_t[:])
            w_sb = pers.tile([128, 3, D_MODEL], f32)
            for qt in range(3):
                nc.sync.dma_start(w_sb[:, qt, :], w0_t[qt * 128 : (qt + 1) * 128, :])

            for l in range(N_LAYER):
                # ---- layer weights ----
                wqkv_sb = wts.tile([128, HP, 3 * D_MODEL], bf16, tag="wqkv")
                wr_sb = wts.tile([128, HP, D_MODEL], bf16, tag="wrl")
                wo_sb = wts.tile([128, HP, D_MODEL], bf16, tag="wol")
                w1_sb = wts.tile([128, HP, D_INNER], bf16, tag="w1l")
                w2_sb = wts.tile([128, 16, D_MODEL], bf16, tag="w2l")
                for d in range(HP):
                    nc.sync.dma_start(wqkv_sb[:, d, :], wqkv_t[l, d * 128 : (d + 1) * 128, :])
                    nc.sync.dma_start(wr_sb[:, d, :], wr_t[l, d * 128 : (d + 1) * 128, :])
                    nc.sync.dma_start(wo_sb[:, d, :], wo_t[l, d * 128 : (d + 1) * 128, :])
                    nc.sync.dma_start(w1_sb[:, d, :], w1_t[l, d * 128 : (d + 1) * 128, :])
                for d in range(16):
                    nc.sync.dma_start(w2_sb[:, d, :], w2_t[l, d * 128 : (d + 1) * 128, :])
                gb_sb = None
                if not trivial_gb:
                    gb_sb = wts.tile([128, 4, D_MODEL], f32, tag="gbl")
                    nc.sync.dma_start(gb_sb[:], gb_t[l])
                b1_sb = b2_sb = None
                if not trivial_b:
                    b1_sb = wts.tile([128, 16], f32, tag="b1l")
                    for d in range(16):
                        nc.sync.dma_start(b1_sb[:, d : d + 1], b1_t[l, d * 128 : (d + 1) * 128, :])
                    b2_sb = wts.tile([128, D_MODEL], f32, tag="b2l")
                    nc.sync.dma_start(b2_sb[:], b2_t[l])

                # ---- transpose residual -> wT bf16 ----
                wT_sb = wts.tile([128, HP, 384], bf16, tag="wT")
                for qt in range(3):
                    for d in range(HP):
                        pt = ps.tile([128, 512], f32, tag="pp")
                        nc.tensor.transpose(
                            pt[:, :128], w_sb[:, qt, d * 128 : (d + 1) * 128], ident[:]
                        )
                        nc.scalar.copy(wT_sb[:, d, qt * 128 : (qt + 1) * 128], pt[:, :128])

                # ---- q/k projections (q pre-scaled by 1/sqrt(d)) ----
                qwT = wts.tile([128, HP, 384], bf16, tag="qwT")
                qrT = wts.tile([128, HP, 384], bf16, tag="qrT")
                kT_own = wts.tile([128, HP, NK_TOK], bf16, tag="kTown")
                for hp in range(HP):
                    pq = ps.tile([128, 512], f32, tag="pp")
                    for d in range(HP):
                        nc.tensor.matmul(
                            pq[:, :384],
                            wqkv_sb[:, d, hp * 128 : hp * 128 + 128],
                            wT_sb[:, d, :],
                            start=(d == 0), stop=(d == HP - 1),
                        )
                    nc.scalar.activation(
                        qwT[:, hp, :], pq[:, :384], AF.Identity,
                        bias=rwb_sb[:, hp : hp + 1], scale=float(SCALE),
                    )
                    nc.scalar.activation(
                        qrT[:, hp, :], pq[:, :384], AF.Identity,
                        bias=rrb_sb[:, hp : hp + 1], scale=float(SCALE),
                    )
                    pk = ps.tile([128, 512], f32, tag="pp")
                    for d in range(HP):
                        nc.tensor.matmul(
                            pk[:, :384],
                            wqkv_sb[:, d, D_MODEL + hp * 128 : D_MODEL + hp * 128 + 128],
                            wT_sb[:, d, :],
                            start=(d == 0), stop=(d == HP - 1),
                        )
                    nc.scalar.copy(kT_own[:, hp, :], pk[:, :NK_TOK])
                    nc.sync.dma_start(
                        agin_k[hp * 128 : (hp + 1) * 128, :], kT_own[:, hp, :]
                    )
                v_own = wts.tile([128, 3, D_MODEL], bf16, tag="vown")
                for qt in range(3):
                    pv = ps.tile([128, 512], f32, tag="pp")
                    for d in range(HP):
                        nc.tensor.matmul(
                            pv[:],
                            wT_sb[:, d, qt * 128 : (qt + 1) * 128],
                            wqkv_sb[:, d, 2 * D_MODEL :],
                            start=(d == 0), stop=(d == HP - 1),
                        )
                    nc.vector.tensor_copy(v_own[:, qt, :], pv[:])
                    rows = 32 if qt == 2 else 128
                    nc.sync.dma_start(
                        agin_v[qt * 128 : qt * 128 + rows, :], v_own[:rows, qt, :]
                    )

                nc.gpsimd.collective_compute(
                    "AllGather", ALU.bypass, replica_groups=rg,
                    ins=[ag_in[:]], outs=[ag_out[:]],
                )

                # ---- rT (posT streamed once per layer) ----
                rT_sb = wts.tile([128, HP, T], bf16, tag="rT")
                for ch in range(5):
                    cw = CH_BIG[ch]
                    c0 = ch * 512
                    pos_ch = posb.tile([128, HP, 512], bf16, tag="posch")
                    for d in range(HP):
                        nc.sync.dma_start(
                            pos_ch[:, d, :cw],
                            posT_t[d * 128 : (d + 1) * 128, c0 : c0 + cw],
                        )
                    for hp in range(HP):
                        pr = ps.tile([128, 512], f32, tag="pp")
                        for d in range(HP):
                            nc.tensor.matmul(
                                pr[:, :cw],
                                wr_sb[:, d, hp * 128 : hp * 128 + 128],
                                pos_ch[:, d, :cw],
                                start=(d == 0), stop=(d == HP - 1),
                            )
                        nc.scalar.copy(rT_sb[:, hp, c0 : c0 + cw], pr[:, :cw])

                # ---- gathered K/V into SBUF ----
                kT_all = kvp.tile([128, HP, T], bf16, tag="kTall")
                v_all = kvp.tile([128, 17, D_MODEL], bf16, tag="vall")
                nc.vector.memset(v_all[:, 16, :], 0.0)
                for r in range(N_CORES):
                    rb, rs = _row_bases(r)
                    srck = ag_out[r, 0:kv_off].rearrange("(a b) -> a b", b=NK_TOK)
                    srcv = ag_out[r, kv_off:].rearrange("(a b) -> a b", b=D_MODEL)
                    for hp in range(HP):
                        nc.sync.dma_start(
                            kT_all[:, hp, rb : rb + 128],
                            srck[hp * 128 : (hp + 1) * 128, 0:128],
                        )
                        nc.sync.dma_start(
                            kT_all[:, hp, rs : rs + 128],
                            srck[hp * 128 : (hp + 1) * 128, 128:256],
                        )
                        if r == 0:
                            nc.sync.dma_start(
                                kT_all[:, hp, 0:32],
                                srck[hp * 128 : (hp + 1) * 128, 256:288],
                            )
                    # v rows: big -> [rb, rb+128), small -> [rs, rs+128), micro r0
                    for base, lo in ((0, rb), (128, rs)):
                        t0_, p0 = lo // 128, lo % 128
                        n1 = 128 - p0
                        nc.sync.dma_start(
                            v_all[p0:128, t0_, :], srcv[base : base + n1, :]
                        )
                        nc.sync.dma_start(
                            v_all[0 : 128 - n1, t0_ + 1, :],
                            srcv[base + n1 : base + 128, :],
                        )
                    if r == 0:
                        nc.sync.dma_start(v_all[0:32, 0, :], srcv[256:288, :])

                # ---- attention ----
                # slot parameters: (qcol0, width, padded width, chunks,
                #                   k-window lo, p-tensor list, probT tile0, n tiles)
                def slot_params(h, si):
                    if si == 0:
                        return (0, W_BIG, PAD_BIG, CH_BIG, 0, p_big[h],
                                128 + pid * 128, 0, 17)
                    if si == 1:
                        return (128, W_SMALL, PAD_SMALL, CH_SMALL, T - W_SMALL,
                                p_small[h], 2048 - pid * 128, 17, 9)
                    return (256, W_MICRO, W_MICRO, [128], T - W_MICRO,
                            p_micro[h], 2080, 26, 1)

                # E computation for one head -> DRAM (decoupled from the
                # rel-shift round trip; interleaved with pass2 below)
                def e_pass(h):
                    hp, par = h // 2, (h % 2) * 64
                    for si in range(3):
                        qc0, wj, wpad, chs, klo, pt_t, off, jt0, njt = slot_params(h, si)
                        p2d = pt_t.rearrange("(r c) -> r c", c=PITCH)
                        qsl = slice(qc0, qc0 + 128)
                        e_sb = epool.tile([128, W_BIG], bf16, tag="esb")
                        cpos = 0
                        for cw in chs:
                            pe = ps.tile([128, 512], f32, tag="pp")
                            nc.tensor.matmul(
                                pe[:, :cw],
                                qrT[par : par + 64, hp, qsl],
                                rT_sb[par : par + 64, hp, klo + cpos : klo + cpos + cw],
                                start=True, stop=True,
                            )
                            nc.scalar.copy(e_sb[:, cpos : cpos + cw], pe[:, :cw])
                            cpos += cw
                        if si == 2:
                            # corner-garbage columns k in [0, 16)
                            pe = ps.tile([128, 512], f32, tag="pp")
                            nc.tensor.matmul(
                                pe[:, :16],
                                qrT[par : par + 64, hp, qsl],
                                rT_sb[par : par + 64, hp, 0:16],
                                start=True, stop=True,
                            )
                            ec = mid.tile([128, 16], bf16, tag="ecrn")
                            nc.scalar.copy(ec[:], pe[:, :16])
                            nc.scalar.dma_start(p2d[0:128, 1:17], ec[:])
                        nc.scalar.dma_start(
                            p2d[0:128, 1 + klo : 1 + klo + wj], e_sb[:, :wj]
                        )

                # pass 2: shifted read-back, scores, softmax, PV.
                # BD reads are prefetched one head ahead so they are never
                # queued behind the probT transposes on the sync queue.
                attnT = wts.tile([128, HP, 384], bf16, tag="attnT")
                BD_TAGS = ["bdb", "bds", "bdm"]
                BD_W = [W_BIG, W_SMALL, W_MICRO]

                def issue_reads(h):
                    bds = []
                    for si in range(3):
                        _, wj, _, _, _, pt_t, off, _, _ = slot_params(h, si)
                        bd_sb = bdp.tile([128, BD_W[si]], bf16, tag=BD_TAGS[si])
                        src_ap = pt_t[ds(off, 128 * T)].rearrange("(a b) -> a b", b=T)
                        nc.sync.dma_start(bd_sb[:, :wj], src_ap[:, :wj])
                        bds.append(bd_sb)
                    return bds

                def pass2(h, bds):
                    hp, par = h // 2, (h % 2) * 64
                    ppv = pspv.tile([64, 384], f32, tag="ppv")
                    for si in range(3):
                        qc0, wj, wpad, chs, klo, pt_t, off, jt0, njt = slot_params(h, si)
                        qsl = slice(qc0, qc0 + 128)
                        bd_sb = bds[si]

                        # scores: AC + BD (+ mask for big/micro; small is
                        # handled entirely by the poison region)
                        if si == 0:
                            nc.vector.tensor_tensor(
                                bd_sb[:, W_BIG - MASKB_W :],
                                bd_sb[:, W_BIG - MASKB_W :], maskb_sb[:], ALU.add,
                            )
                        elif si == 2:
                            nc.vector.tensor_tensor(
                                bd_sb[:, :wj], bd_sb[:, :wj], maskm_sb[:, :wj], ALU.add
                            )
                        cpos = 0
                        for cw in chs:
                            jsl = slice(cpos, cpos + cw)
                            pa = ps.tile([128, 512], f32, tag="pp")
                            nc.tensor.matmul(
                                pa[:, :cw],
                                qwT[par : par + 64, hp, qsl],
                                kT_all[par : par + 64, hp, jsl],
                                start=True, stop=True,
                            )
                            nc.vector.scalar_tensor_tensor(
                                bd_sb[:, jsl], pa[:, :cw], 1.0,
                                bd_sb[:, jsl], ALU.mult, ALU.add,
                            )
                            cpos += cw

                        # softmax over computed j range
                        prob = probp.tile([128, wpad], bf16, tag=f"prob{si}")
                        denom = sm.tile([128, 1], f32, tag="denom")
                        if wpad > wj:
                            nc.vector.memset(prob[:, wj:wpad], 0.0)
                        nc.scalar.activation(
                            prob[:, :wj], bd_sb[:, :wj], AF.Exp,
                            bias=0.0, scale=1.0, accum_out=denom[:, :],
                        )
                        rden = sm.tile([128, 1], f32, tag="rden")
                        nc.vector.reciprocal(rden[:], denom[:])
                        nc.vector.tensor_scalar(
                            out=prob[:], in0=prob[:],
                            scalar1=rden[:], scalar2=None, op0=ALU.mult,
                        )
                        probT = ptp.tile([128, njt, 128], bf16, tag=f"probT{si}")
                        if si == 0:
                            nc.sync.dma_start_transpose(probT[:], prob[:])
                        else:
                            nc.scalar.dma_start_transpose(probT[:], prob[:])
                        # PV for this slot
                        for t in range(njt):
                            nc.tensor.matmul(
                                ppv[:, qc0 : qc0 + 128],
                                v_all[:, t, h * 64 : h * 64 + 64],
                                probT[:, t, :],
                                start=(t == 0), stop=(t == njt - 1),
                            )
                    nc.scalar.copy(attnT[par : par + 64, hp, :], ppv[:])

                for h in range(N_HEAD):
                    e_pass(h)
                prev = issue_reads(0)
                for h in range(N_HEAD):
                    nxt = issue_reads(h + 1) if h + 1 < N_HEAD else None
                    pass2(h, prev)
                    prev = nxt

                # ---- Wo + residual + LN1 ----
                for qt in range(3):
                    pw = ps.tile([128, 512], f32, tag="pp")
                    for d in range(HP):
                        nc.tensor.matmul(
                            pw[:],
                            attnT[:, d, qt * 128 : (qt + 1) * 128],
                            wo_sb[:, d, :],
                            start=(d == 0), stop=(d == HP - 1),
                        )
                    x = sm.tile([128, D_MODEL], f32, tag="xres")
                    nc.vector.tensor_tensor(x[:], w_sb[:, qt, :], pw[:], ALU.add)
                    _layernorm(
                        nc, sm, w_sb[:, qt, :], x,
                        None if trivial_gb else gb_sb[:, 0, :],
                        None if trivial_gb else gb_sb[:, 1, :],
                        eps_sb[:],
                    )

                # ---- FFN ----
                w1T = wts.tile([128, HP, 384], bf16, tag="wT")
                for qt in range(3):
                    for d in range(HP):
                        pt = ps.tile([128, 512], f32, tag="pp")
                        nc.tensor.transpose(
                            pt[:, :128], w_sb[:, qt, d * 128 : (d + 1) * 128], ident[:]
                        )
                        nc.scalar.copy(w1T[:, d, qt * 128 : (qt + 1) * 128], pt[:, :128])
                pf = [
                    psff.tile([128, 512], f32, tag=f"pf{qt}", name=f"pf{qt}")
                    for qt in range(3)
                ]
                for di in range(16):
                    phh = ps.tile([128, 512], f32, tag="pp")
                    for d in range(HP):
                        nc.tensor.matmul(
                            phh[:, :384],
                            w1_sb[:, d, di * 128 : (di + 1) * 128],
                            w1T[:, d, :],
                            start=(d == 0), stop=(d == HP - 1),
                        )
                    h1t = mid.tile([128, 384], bf16, tag="h1t")
                    if trivial_b:
                        nc.scalar.activation(
                            h1t[:], phh[:, :384], AF.Relu, bias=0.0, scale=1.0
                        )
                    else:
                        nc.scalar.activation(
                            h1t[:], phh[:, :384], AF.Relu,
                            bias=b1_sb[:, di : di + 1], scale=1.0,
                        )
                    for qt in range(3):
                        nc.tensor.matmul(
                            pf[qt][:],
                            h1t[:, qt * 128 : (qt + 1) * 128],
                            w2_sb[:, di, :],
                            start=(di == 0), stop=(di == 15),
                        )
                for qt in range(3):
                    x = sm.tile([128, D_MODEL], f32, tag="xres")
                    if trivial_b:
                        nc.vector.tensor_tensor(x[:], pf[qt][:], w_sb[:, qt, :], ALU.add)
                    else:
                        nc.vector.scalar_tensor_tensor(
                            x[:], pf[qt][:], 1.0, b2_sb[:], ALU.mult, ALU.add
                        )
                        nc.vector.tensor_tensor(x[:], x[:], w_sb[:, qt, :], ALU.add)
                    _layernorm(
                        nc, sm, w_sb[:, qt, :], x,
                        None if trivial_gb else gb_sb[:, 2, :],
                        None if trivial_gb else gb_sb[:, 3, :],
                        eps_sb[:],
                    )

            for qt in range(3):
                nc.sync.dma_start(
                    out_t[qt * 128 : (qt + 1) * 128, :], w_sb[:, qt, :]
                )

    nc.compile()
    return nc


_NC_CACHE = {}
LAST_RESULT = None


def kernel(**inputs):
    global LAST_RESULT
    trivial_gb = (
        np.all(np.asarray(inputs["ln1_scale"]) == 1.0)
        and np.all(np.asarray(inputs["ln2_scale"]) == 1.0)
        and np.all(np.asarray(inputs["ln1_bias"]) == 0.0)
        and np.all(np.asarray(inputs["ln2_bias"]) == 0.0)
    )
    trivial_b = (
        np.all(np.asarray(inputs["ffn_b1"]) == 0.0)
        and np.all(np.asarray(inputs["ffn_b2"]) == 0.0)
    )
    per_core = _host_prep(inputs)
    drop = []
    if trivial_gb:
        drop.append("gb")
    if trivial_b:
        drop += ["b1col", "b2bc"]
    for pc in per_core:
        for k in drop:
            pc.pop(k, None)
    key = (trivial_gb, trivial_b)
    if key not in _NC_CACHE:
        _NC_CACHE[key] = _build(trivial_gb=trivial_gb, trivial_b=trivial_b)
    res = run_bass_kernel_spmd(
        _NC_CACHE[key], [dict(pc) for pc in per_core], core_ids=list(range(N_CORES)),
        tmpdir=os.environ.get("BASS_TMPDIR") or None,
    )
    LAST_RESULT = res
    out = np.zeros((T, D_MODEL), np.float32)
    for m in range(N_CORES):
        rb, rs = _row_bases(m)
        wout = res.results[m]["wout"]
        out[rb : rb + 128] = wout[0:128]
        out[rs : rs + 128] = wout[128:256]
        if m == 0:
            out[0:32] = wout[256:288]
    return np.ascontiguousarray(out[:, None, :].astype(np.float32))
